# revision 1
# baseline (speedup 1.0000x reference)
"""Trainium2 Bass kernel for MultiHeadSelfAttention with relative position
embeddings (Transformer-XL style), B=2, T=512, D=512, H=8.

Sharding: pure data/sequence parallel — core c owns batch b=c//4 and query
rows i in [128*(c%4), 128*(c%4)+128). Every core's output slice is disjoint,
so there are no collectives.

Key algebraic restructuring: pos = rel @ Wp (274 GFLOP) is never formed.
Since pos_score[h,i,j] = sum_d q_v[h,i,d] * (rel[i,j] @ Wp + bp)[h,d], we
fold q_v into Wp per query row:  r_i[c,h] = sum_hd Wp[c, h*64+hd] q_v[h,i,hd]
then pos_score[h,i,j] = sum_c rel[i,j,c] r_i[c,h] + (bp . q_v[h,i]).
rel is streamed from HBM exactly once -> DMA-bound kernel.

Division of labor: the O(T^2 D) stream (pos scores), the qk scores, softmax,
context and out-projection run on device. The tiny O(T D^2) linear
projections (q/k/v and the Wp fold, ~1 GFLOP of numpy total) run on the
host, which removes the on-chip prologue dependency chain entirely: the
device starts consuming rel within a few microseconds.

Layouts/dtypes: rel is host-transposed to [e, i, j] bf16 (contraction dim e
on partitions -> no on-chip transposes, half the HBM bytes). Scores live in
S^T layout [j, (h,i)] bf16; pos scores accumulate in fp32 PSUM and are
folded in via a stack+transpose pipeline deferred one group so SWDGE
latency hides behind streaming.
"""

import math
import os
import numpy as np
import ml_dtypes

import concourse.bacc as bacc
import concourse.bass as bass
import concourse.mybir as mybir
import concourse.tile as tile
from concourse.bass_utils import run_bass_kernel_spmd
from concourse.masks import make_identity

B, T, D, H = 2, 512, 512, 8
HD = D // H          # 64
I = 128              # query rows per core
GI = 8               # query rows per rel DMA group
N_CORES = 8
F32 = mybir.dt.float32
F32R = mybir.dt.float32r
BF16 = mybir.dt.bfloat16

_CACHED = {}

_PHASES = ("qk", "grp1", "grp4", "loop", "sums", "ctx", "full")


def _build_nc(phase=None):
    phase = phase or os.environ.get("KPHASE", "full")
    lvl = _PHASES.index(phase)
    nc = bacc.Bacc("TRN2", target_bir_lowering=False, debug=False)

    # ---- DRAM I/O (per-core shards), all host-prepacked ----
    # rel: [ec, p, i, j] bf16 with e = ec*128+p (8 KB runs per (p, ec, grp))
    rel = nc.dram_tensor("rel", [4, 128, I, T], BF16, kind="ExternalInput")
    # r = SC * (Wp.T-folded q_v): [ct, c', i*8+h] bf16
    rdr = nc.dram_tensor("r", [4, 128, I * 8], BF16, kind="ExternalInput")
    # kT packed [p, (dm, j)]: row dm*128+p of (x@Wk+bk).T
    ktp = nc.dram_tensor("ktp", [128, 4 * T], BF16, kind="ExternalInput")
    # v packed [p, (jm, h, 72)]: token row jm*128+p of x@Wv+bv, with 8
    # ones-columns appended per head so one matmul per (h, jt) yields both
    # the context contribution (cols 0-63) and the softmax sum (cols 64-71)
    vp = nc.dram_tensor("vp", [128, 4 * 8 * 72], BF16, kind="ExternalInput")
    # quT packed [p, (dm, i)]: row dm*128+p of ((xi@Wq+bq+u)*SC).T
    qup = nc.dram_tensor("qup", [128, 4 * I], BF16, kind="ExternalInput")
    # wo packed [p, (kc, d)]
    wo = nc.dram_tensor("wo", [128, 4 * D], BF16, kind="ExternalInput")
    bo = nc.dram_tensor("bo", [D], F32, kind="ExternalInput")
    out = nc.dram_tensor("out", [I, D], F32, kind="ExternalOutput")

    with tile.TileContext(nc) as tc:
        with (
            tc.tile_pool(name="spool", bufs=1) as spool,
            tc.tile_pool(name="rel_p", bufs=4) as rel_p,
            tc.tile_pool(name="stg_p", bufs=4) as stg_p,
            tc.tile_pool(name="psA", bufs=2, space="PSUM") as psA,
            tc.tile_pool(name="psB", bufs=4, space="PSUM") as psB,
            tc.tile_pool(name="psC", bufs=2, space="PSUM") as psC,
        ):
            # ---------- constants + inputs ----------
            # r first: it is the only dependency of the streaming loop.
            r_sb = [spool.tile([128, I * 8], BF16, tag=f"r{ct}",
                               name=f"r{ct}") for ct in range(4)]
            for ct in range(4):
                eng = nc.sync if ct % 2 == 0 else nc.scalar
                eng.dma_start(out=r_sb[ct], in_=rdr[ct])

            kT_t = spool.tile([128, 4 * T], BF16, tag="ktp")
            nc.sync.dma_start(out=kT_t, in_=ktp[:, :])
            qu_t = spool.tile([128, 4 * I], BF16, tag="qup")
            nc.scalar.dma_start(out=qu_t, in_=qup[:, :])
            v_t = spool.tile([128, 4 * 8 * 72], BF16, tag="vp")
            nc.sync.dma_start(out=v_t, in_=vp[:, :])
            wo_t = spool.tile([128, 4 * D], BF16, tag="wo")
            nc.scalar.dma_start(out=wo_t, in_=wo[:, :])

            ident_f = spool.tile([128, 128], F32)
            make_identity(nc, ident_f)
            ident = spool.tile([128, 128], F32R)
            nc.vector.tensor_copy(ident, ident_f)
            def bcast_ap(handle):
                a = handle[:]
                return bass.AP(tensor=a.tensor, offset=a.offset,
                               ap=[[0, 128]] + list(a.ap))

            bo_bc = spool.tile([128, D], F32, tag="bo_bc")
            nc.sync.dma_start(out=bo_bc, in_=bcast_ap(bo))

            # ---------- qk scores into sT_int (S^T layout, bf16) ----------
            # h-major cols (h*128 + i): matmul lhsT slices must be
            # contiguous — strided-AP weights crash the PE.
            sT_int = [spool.tile([128, I * 8], BF16, tag=f"sT{jt}",
                                 name=f"sT{jt}") for jt in range(4)]
            for h in range(8):
                dm, po = h // 2, (h % 2) * 64
                for jt in range(4):
                    ps = psA.tile([128, 128], F32, tag="pt",
                                  name=f"ps_qk{h}_{jt}")
                    nc.tensor.matmul(
                        ps,
                        lhsT=kT_t[po:po + 64,
                                  dm * T + jt * 128:dm * T + (jt + 1) * 128],
                        rhs=qu_t[po:po + 64, dm * I:(dm + 1) * I],
                        start=True, stop=True,
                    )
                    dst = sT_int[jt][:, h * 128:(h + 1) * 128]
                    eng = (nc.vector.tensor_copy if h % 2 == 0
                           else nc.scalar.copy)
                    eng(dst, ps)

            if lvl == 0:   # qk
                dbg = spool.tile([128, 512], F32, tag="dbg")
                nc.vector.tensor_copy(dbg, sT_int[0][:, 0:512])
                nc.sync.dma_start(out=out[:, :], in_=dbg)

            # ---------- main loop over query rows ----------
            # per 4-row bank: 16 col-tiled matmuls into one PSUM bank, one
            # whole-bank copy to SBUF staging, then 4 PE transposes of the
            # staged tile (padded (32k+h) columns) and strided adds into
            # sT_int. No stack, no SWDGE DMAs — the only cross-engine dep
            # is the staging copy, deferred one bank so the PE never waits.
            def process_bank(bk, stg):
                ps_t = psC.tile([128, 512], F32, tag="ps_s",
                                name=f"ps_t{bk}")
                for jt in range(4):
                    nc.tensor.transpose(
                        out=ps_t[:, jt * 128:(jt + 1) * 128],
                        in_=stg[:, jt * 128:(jt + 1) * 128],
                        identity=ident_f,
                    )
                # ps_t cols are (k, c) = k*32 + h (c>=8 garbage); sT_int
                # cols are (h, i) with i = bk*4+k.
                for jt in range(4):
                    sl = sT_int[jt].rearrange(
                        "p (h i) -> p h i", h=8)[:, :, bk * 4:(bk + 1) * 4]
                    nc.vector.tensor_tensor(
                        sl, sl,
                        ps_t[:, jt * 128:(jt + 1) * 128].rearrange(
                            "p (k c) -> p c k", k=4)[:, 0:8, :],
                        op=mybir.AluOpType.add)

            def exp_grp(grp):
                for jt in range(4):
                    sl = sT_int[jt].rearrange(
                        "p (h i) -> p h i", h=8)[:, :, grp * 16:(grp + 1) * 16]
                    nc.scalar.activation(sl, sl,
                                         mybir.ActivationFunctionType.Exp)

            n_grp = {0: 0, 1: 1, 2: 4}.get(lvl, 8)
            pending = None
            for bk in range(n_grp * 4):
                g, bank = bk // 2, bk % 2
                if bank == 0:
                    # one consolidated bf16 DMA per 8 rows: [p, (ec, i, j)],
                    # per (partition, ec) an 8 KB contiguous run
                    relg = rel_p.tile([128, 4 * GI * T], BF16, tag="rel",
                                      name=f"rel{g}")
                    eng = nc.sync if g % 2 == 0 else nc.scalar
                    eng.dma_start(
                        out=relg.rearrange("p (ec i j) -> p ec i j",
                                           ec=4, i=GI),
                        in_=rel[:, :, g * GI:(g + 1) * GI, :].rearrange(
                            "ec p i j -> p ec i j"),
                    )
                # 4 query rows go to the PE's 4 column-groups
                # (tile_position col-tiling): their rhs streams run
                # concurrently, ~4x less PE wall time per bank.
                ps_pos = psB.tile([128, 512], F32, tag="pos",
                                  name=f"ps_pos{bk}")
                for ct in range(4):
                    for k in range(4):
                        i = bk * 4 + k
                        col = (ct * GI + bank * 4 + k) * T
                        nc.tensor.matmul(
                            ps_pos[32 * k:32 * k + 8, :],
                            lhsT=r_sb[ct][:, i * 8:(i + 1) * 8],
                            rhs=relg[:, col:col + T],
                            start=(ct == 0), stop=(ct == 3),
                            tile_position=(0, 32 * k),
                        )
                # stage the whole bank in ONE copy (engine cost is
                # per-partition bytes, so [128,512] costs same as [8,512])
                stg = stg_p.tile([128, 512], F32, tag="stg",
                                 name=f"stg{bk}")
                eng = (nc.vector.tensor_copy if bk % 2 == 0
                       else nc.scalar.copy)
                eng(stg, ps_pos)
                if pending is not None:
                    process_bank(*pending)
                    if pending[0] % 4 == 3:
                        exp_grp(pending[0] // 4)
                pending = (bk, stg)
            if pending is not None:
                process_bank(*pending)
                if pending[0] % 4 == 3:
                    exp_grp(pending[0] // 4)

            if 1 <= lvl <= 3:   # grp1/grp4/loop
                dbg = spool.tile([128, 512], F32, tag="dbg")
                nc.vector.tensor_copy(dbg, sT_int[0][:, 0:512])
                nc.sync.dma_start(out=out[:, :], in_=dbg)

            if lvl >= 4:
                # ------- fused context + softmax sums (shared lhsT) -------
                # one matmul per (h, jt): rhs = [v-block | ones8] so cols
                # 0-63 accumulate context and 64-71 the softmax sum; then
                # per-partition reciprocal + scalar-mul normalize in the
                # PSUM epilogue.
                ps_cs = [psB.tile([128, 512], F32, tag="pos",
                                  name=f"ps_cs{half}") for half in range(2)]
                for h in range(8):
                    dst = ps_cs[h // 4][:, (h % 4) * 72:(h % 4 + 1) * 72]
                    for jt in range(4):
                        nc.tensor.matmul(
                            dst,
                            lhsT=sT_int[jt][:, h * 128:(h + 1) * 128],
                            rhs=v_t[:, jt * 576 + h * 72:jt * 576 + (h + 1) * 72],
                            start=(jt == 0), stop=(jt == 3),
                        )
                inv_ih = spool.tile([128, 64], F32, tag="inv_ih")
                for h in range(8):
                    nc.vector.reciprocal(
                        inv_ih[:, h * 8:(h + 1) * 8],
                        ps_cs[h // 4][:, (h % 4) * 72 + 64:(h % 4) * 72 + 72])

                if lvl == 4:   # sums
                    dbg = spool.tile([128, 512], F32, tag="dbg")
                    nc.vector.tensor_copy(dbg[:, 0:64], inv_ih)
                    nc.vector.tensor_copy(dbg[:, 64:128], inv_ih)
                    nc.vector.memset(dbg[:, 128:512], 0.0)
                    nc.sync.dma_start(out=out[:, :], in_=dbg)

            if lvl >= 5:
                ctx_sb = spool.tile([128, 512], F32R, tag="ctx")
                for h in range(8):
                    nc.vector.tensor_scalar_mul(
                        ctx_sb[:, h * 64:(h + 1) * 64],
                        ps_cs[h // 4][:, (h % 4) * 72:(h % 4) * 72 + 64],
                        inv_ih[:, h * 8:h * 8 + 1])
                if lvl == 5:   # ctx
                    dbg = spool.tile([128, 512], F32, tag="dbg")
                    nc.vector.tensor_copy(dbg, ctx_sb)
                    nc.sync.dma_start(out=out[:, :], in_=dbg)

            if lvl >= 6:
                # ctxT
                ps_ct = psC.tile([128, 512], F32R, tag="ps_s", name="ps_ct")
                for dt_ in range(4):
                    nc.tensor.transpose(
                        out=ps_ct[:, dt_ * 128:(dt_ + 1) * 128],
                        in_=ctx_sb[:, dt_ * 128:(dt_ + 1) * 128],
                        identity=ident,
                    )
                ctxT_sb = spool.tile([128, 512], BF16, tag="ctxT")
                nc.vector.tensor_copy(ctxT_sb, ps_ct)
                # out projection
                ps_o = psB.tile([128, 512], F32, tag="pos", name="ps_o")
                for dt_ in range(4):
                    nc.tensor.matmul(
                        ps_o,
                        lhsT=ctxT_sb[:, dt_ * 128:(dt_ + 1) * 128],
                        rhs=wo_t[:, dt_ * D:(dt_ + 1) * D],
                        start=(dt_ == 0), stop=(dt_ == 3),
                    )
                out_sb = spool.tile([128, 512], F32, tag="out_sb")
                nc.vector.tensor_tensor(out_sb, ps_o, bo_bc,
                                        op=mybir.AluOpType.add)
                nc.sync.dma_start(out=out[:, :], in_=out_sb)

    nc.compile()
    return nc


def kernel(**inputs):
    inputs = {k: np.asarray(v) for k, v in inputs.items()}
    x = np.ascontiguousarray(inputs["inputs"], dtype=np.float32)      # [B, T, D]
    rel = inputs["rel_pos_emb"]                                        # [B, T, T, D]
    if rel.dtype != np.float32:
        rel = rel.astype(np.float32)
    f32 = lambda a: np.ascontiguousarray(a, dtype=np.float32)
    Wq, Wk, Wv, Wp, Wo = (f32(inputs[k]) for k in ("Wq", "Wk", "Wv", "Wp", "Wo"))
    bq, bk, bv, bp, bo = (f32(inputs[k]) for k in ("bq", "bk", "bv", "bp", "bo"))
    u = f32(inputs["u_bias"]).reshape(-1)
    v = f32(inputs["v_bias"]).reshape(-1)

    if "nc" not in _CACHED:
        _CACHED["nc"] = _build_nc()
    nc = _CACHED["nc"]

    SC = 1.0 / math.sqrt(HD)
    bf16 = ml_dtypes.bfloat16

    def pack(w, ncol):
        # [rows, ncol] -> [p, (chunk, ncol)]: chunk-of-128-rows packing so
        # each tensor loads as a single long-run DMA
        return np.ascontiguousarray(
            np.asarray(w, np.float32).astype(bf16).reshape(
                -1, 128, ncol).transpose(1, 0, 2)).reshape(128, -1)

    # host-side projections (~1 GFLOP of numpy total)
    q_v = x @ Wq + bq + v                                # [B, T, D]
    q_u = (x @ Wq + bq + u) * SC
    k_all = x @ Wk + bk
    v_all = x @ Wv + bv
    Wp4 = Wp.reshape(D, H, HD)
    r_all = np.einsum("chd,bihd->bcih", Wp4,
                      q_v.reshape(B, T, H, HD) * SC).astype(bf16)
    wo_b = pack(Wo, D)

    in_maps = []
    for c in range(N_CORES):
        b, blk = c // 4, c % 4
        # rel shard: [128i, 512j, 512e] f32 -> [4ec, 128p, 128i, 512j] bf16
        shard = rel[b, blk * I:(blk + 1) * I].astype(bf16)
        shard = np.ascontiguousarray(shard.transpose(2, 0, 1)).reshape(
            4, 128, I, T)
        r_shard = np.ascontiguousarray(
            r_all[b, :, blk * I:(blk + 1) * I, :]).reshape(4, 128, I * 8)
        # v with 8 ones-columns per head: [4jm, 128p, 8h, 72]
        v4 = v_all[b].reshape(4, 128, H, HD)
        vo = np.concatenate(
            [v4, np.ones((4, 128, H, 8), np.float32)], axis=3)
        vp_b = np.ascontiguousarray(
            vo.astype(bf16).transpose(1, 0, 2, 3)).reshape(128, 4 * 8 * 72)
        in_maps.append({
            "rel": shard,
            "r": r_shard,
            "ktp": pack(k_all[b].T, T),
            "vp": vp_b,
            "qup": pack(q_u[b, blk * I:(blk + 1) * I].T, I),
            "wo": wo_b,
            "bo": bo,
        })

    res = run_bass_kernel_spmd(nc, in_maps, list(range(N_CORES)),
                               trace=bool(os.environ.get("KBENCH_TRACE")),
                               tmpdir=os.environ.get("KBENCH_TMPDIR"))
    out = np.empty((B, T, D), np.float32)
    for c in range(N_CORES):
        b, blk = c // 4, c % 4
        out[b, blk * I:(blk + 1) * I] = res.results[c]["out"]
    if os.environ.get("KBENCH_TRACE"):
        _CACHED["last_exec_time_ns"] = res.exec_time_ns
        _CACHED["last_mean_exec_time_ns"] = res.mean_exec_time_ns
    return out



# revision 3
# speedup vs baseline: 1.6231x; 1.6231x over previous
"""Trainium2 Bass kernel for MultiHeadSelfAttention with relative position
embeddings (Transformer-XL style), B=2, T=512, D=512, H=8.

Sharding: pure data/sequence parallel — core c owns batch b=c//4 and query
rows i in [128*(c%4), 128*(c%4)+128). Every core's output slice is disjoint,
so there are no collectives.

Key algebraic restructuring: pos = rel @ Wp (274 GFLOP) is never formed.
Since pos_score[h,i,j] = sum_d q_v[h,i,d] * (rel[i,j] @ Wp + bp)[h,d], we
fold q_v into Wp per query row:  r_i[c,h] = sum_hd Wp[c, h*64+hd] q_v[h,i,hd]
then pos_score[h,i,j] = sum_c rel[i,j,c] r_i[c,h] + (bp . q_v[h,i]).
rel is streamed from HBM exactly once -> DMA-bound kernel.

fp8 stream: rel is host-quantized to float8_e3m4 (4 mantissa bits; RMS
quantization error ~1.3% of a unit-normal element) which HALVES the HBM
traffic vs bf16 — 32 MiB/core instead of 64. The pos matmul runs mixed
precision: bf16 stationary (r) x fp8 moving (rel); the PE upconverts both
operands, so only rel carries fp8 noise. End-to-end rel err ~1.4e-2 vs the
2e-2 gate (verified against the jax reference). r itself must stay bf16:
its values (~N(0, 0.04^2)) sit inside e3m4's subnormal range.

Division of labor: the O(T^2 D) stream (pos scores), the qk scores, softmax,
context and out-projection run on device. The tiny O(T D^2) linear
projections (q/k/v and the Wp fold, ~1 GFLOP of numpy total) run on the
host, which removes the on-chip prologue dependency chain entirely.

Layouts: rel is host-packed to [p][g][i][ec][j] so each 2 MiB group DMA is
a single 16 KB-contiguous run per partition. Scores live in S^T layout
[j, (h,i)] bf16; pos scores accumulate in fp32 PSUM and are folded in via a
stage+transpose pipeline deferred one bank so latency hides behind the
stream. The prologue (r fold + k/q/v/wo constants) is packed into 2 bf16
DMAs issued on the scalar HWDGE queue ahead of the odd rel groups; even rel
groups stream on the (otherwise idle) sync queue from t=0.
"""

import math
import os
import numpy as np
import ml_dtypes

import concourse.bacc as bacc
import concourse.bass as bass
import concourse.mybir as mybir
import concourse.tile as tile
from concourse.bass_utils import run_bass_kernel_spmd
from concourse.masks import make_identity

B, T, D, H = 2, 512, 512, 8
HD = D // H          # 64
I = 128              # query rows per core
GI = 8               # query rows per rel DMA group
NG = I // GI         # 16 rel DMA groups
N_CORES = 8
F32 = mybir.dt.float32
F32R = mybir.dt.float32r
BF16 = mybir.dt.bfloat16
FP8 = mybir.dt.float8e3

# column offsets inside the packed constant tensor cw [128, 6912]
KT0, QU0, V0, WO0 = 0, 2048, 2560, 4864
CW_COLS = 6912

_CACHED = {}

_PHASES = ("qk", "grp1", "grp4", "loop", "sums", "ctx", "full")


def _build_nc(phase=None):
    phase = phase or os.environ.get("KPHASE", "full")
    lvl = _PHASES.index(phase)
    nc = bacc.Bacc("TRN2", target_bir_lowering=False, debug=False)

    # ---- DRAM I/O (per-core shards), all host-prepacked ----
    # rel: [p, g, (i, ec, j)] fp8e3 with c = ec*128+p, i = g*GI+i'
    # (16 KB contiguous per (p, g) -> one long-run DMA per group)
    rel = nc.dram_tensor("rel", [128, NG, GI * 4 * T], FP8, kind="ExternalInput")
    # r = SC * (Wp.T-folded q_v): [p, (ct, i*8+h)] bf16
    rdr = nc.dram_tensor("r", [128, 4 * I * 8], BF16, kind="ExternalInput")
    # packed constants: kT [p,(dm,j)] | quT [p,(dm,i)] | v+ones [p,(jm,h,72)]
    # | wo [p,(kc,d)]
    cw = nc.dram_tensor("cw", [128, CW_COLS], BF16, kind="ExternalInput")
    bo = nc.dram_tensor("bo", [D], F32, kind="ExternalInput")
    out = nc.dram_tensor("out", [I, D], F32, kind="ExternalOutput")

    with tile.TileContext(nc) as tc:
        with (
            tc.tile_pool(name="spool", bufs=1) as spool,
            tc.tile_pool(name="rel_p", bufs=6) as rel_p,
            tc.tile_pool(name="stg_p", bufs=4) as stg_p,
            tc.tile_pool(name="psA", bufs=2, space="PSUM") as psA,
            tc.tile_pool(name="psB", bufs=4, space="PSUM") as psB,
            tc.tile_pool(name="psC", bufs=2, space="PSUM") as psC,
        ):
            # ---------- prologue loads (scalar HWDGE queue) ----------
            # r first: it is the only dependency of the streaming loop.
            r_sb = spool.tile([128, 4 * I * 8], BF16, tag="r", name="r")
            nc.scalar.dma_start(out=r_sb, in_=rdr[:, :])
            cw_t = spool.tile([128, CW_COLS], BF16, tag="cw", name="cw")
            nc.scalar.dma_start(out=cw_t, in_=cw[:, :])

            ident_f = spool.tile([128, 128], F32)
            make_identity(nc, ident_f)
            ident = spool.tile([128, 128], F32R)
            nc.vector.tensor_copy(ident, ident_f)

            def bcast_ap(handle):
                a = handle[:]
                return bass.AP(tensor=a.tensor, offset=a.offset,
                               ap=[[0, 128]] + list(a.ap))

            bo_bc = spool.tile([128, D], F32, tag="bo_bc")
            nc.scalar.dma_start(out=bo_bc, in_=bcast_ap(bo))

            # ---------- qk scores into sT_int (S^T layout, bf16) ----------
            # h-major cols (h*128 + i): matmul lhsT slices must be
            # contiguous — strided-AP weights crash the PE.
            sT_int = [spool.tile([128, I * 8], BF16, tag=f"sT{jt}",
                                 name=f"sT{jt}") for jt in range(4)]
            for h in range(8):
                dm, po = h // 2, (h % 2) * 64
                for jt in range(4):
                    ps = psA.tile([128, 128], F32, tag="pt",
                                  name=f"ps_qk{h}_{jt}")
                    nc.tensor.matmul(
                        ps,
                        lhsT=cw_t[po:po + 64,
                                  KT0 + dm * T + jt * 128:
                                  KT0 + dm * T + (jt + 1) * 128],
                        rhs=cw_t[po:po + 64, QU0 + dm * I:QU0 + (dm + 1) * I],
                        start=True, stop=True,
                    )
                    dst = sT_int[jt][:, h * 128:(h + 1) * 128]
                    eng = (nc.vector.tensor_copy if h % 2 == 0
                           else nc.scalar.copy)
                    eng(dst, ps)

            if lvl == 0:   # qk
                dbg = spool.tile([128, 512], F32, tag="dbg")
                nc.vector.tensor_copy(dbg, sT_int[0][:, 0:512])
                nc.sync.dma_start(out=out[:, :], in_=dbg)

            # ---------- main loop over query rows ----------
            # per 4-row bank: 16 col-tiled matmuls into one PSUM bank, one
            # whole-bank copy to SBUF staging, then 4 PE transposes of the
            # staged tile (padded (32k+h) columns) and strided adds into
            # sT_int. The only cross-engine dep is the staging copy,
            # deferred one bank so the PE never waits.
            def process_bank(bk, stg):
                ps_t = psC.tile([128, 512], F32, tag="ps_s",
                                name=f"ps_t{bk}")
                for jt in range(4):
                    nc.tensor.transpose(
                        out=ps_t[:, jt * 128:(jt + 1) * 128],
                        in_=stg[:, jt * 128:(jt + 1) * 128],
                        identity=ident_f,
                    )
                # ps_t cols are (k, c) = k*32 + h (c>=8 garbage); sT_int
                # cols are (h, i) with i = bk*4+k.
                for jt in range(4):
                    sl = sT_int[jt].rearrange(
                        "p (h i) -> p h i", h=8)[:, :, bk * 4:(bk + 1) * 4]
                    nc.vector.tensor_tensor(
                        sl, sl,
                        ps_t[:, jt * 128:(jt + 1) * 128].rearrange(
                            "p (k c) -> p c k", k=4)[:, 0:8, :],
                        op=mybir.AluOpType.add)

            def exp_grp(grp):
                for jt in range(4):
                    sl = sT_int[jt].rearrange(
                        "p (h i) -> p h i", h=8)[:, :, grp * 16:(grp + 1) * 16]
                    nc.scalar.activation(sl, sl,
                                         mybir.ActivationFunctionType.Exp)

            n_grp = {0: 0, 1: 1, 2: 4}.get(lvl, NG)
            pending = None
            for bk in range(n_grp * 2):
                g, bank = bk // 2, bk % 2
                if bank == 0:
                    # one 2 MiB DMA per 8 rows: per partition a single
                    # 16 KB contiguous run on both HBM and SBUF side
                    relg = rel_p.tile([128, GI * 4 * T], FP8, tag="rel",
                                      name=f"rel{g}")
                    eng = nc.sync if g % 2 == 0 else nc.scalar
                    eng.dma_start(out=relg, in_=rel[:, g])
                # 4 query rows go to the PE's 4 column-groups
                # (tile_position col-tiling): their rhs streams run
                # concurrently, ~4x less PE wall time per bank.
                ps_pos = psB.tile([128, 512], F32, tag="pos",
                                  name=f"ps_pos{bk}")
                for ct in range(4):
                    for k in range(4):
                        i = bk * 4 + k
                        il = bank * 4 + k          # row within group
                        col = (il * 4 + ct) * T
                        nc.tensor.matmul(
                            ps_pos[32 * k:32 * k + 8, :],
                            lhsT=r_sb[:, ct * 1024 + i * 8:
                                      ct * 1024 + (i + 1) * 8],
                            rhs=relg[:, col:col + T],
                            start=(ct == 0), stop=(ct == 3),
                            tile_position=(0, 32 * k),
                        )
                # stage the whole bank in ONE copy (engine cost is
                # per-partition bytes, so [128,512] costs same as [8,512])
                stg = stg_p.tile([128, 512], F32, tag="stg",
                                 name=f"stg{bk}")
                eng = (nc.vector.tensor_copy if bk % 2 == 0
                       else nc.scalar.copy)
                eng(stg, ps_pos)
                if pending is not None:
                    process_bank(*pending)
                    if pending[0] % 4 == 3:
                        exp_grp(pending[0] // 4)
                pending = (bk, stg)
            if pending is not None:
                process_bank(*pending)
                if pending[0] % 4 == 3:
                    exp_grp(pending[0] // 4)

            if 1 <= lvl <= 3:   # grp1/grp4/loop
                dbg = spool.tile([128, 512], F32, tag="dbg")
                nc.vector.tensor_copy(dbg, sT_int[0][:, 0:512])
                nc.sync.dma_start(out=out[:, :], in_=dbg)

            if lvl >= 4:
                # ------- fused context + softmax sums (shared lhsT) -------
                # one matmul per (h, jt): rhs = [v-block | ones8] so cols
                # 0-63 accumulate context and 64-71 the softmax sum; then
                # per-partition reciprocal + scalar-mul normalize in the
                # PSUM epilogue.
                ps_cs = [psB.tile([128, 512], F32, tag="pos",
                                  name=f"ps_cs{half}") for half in range(2)]
                for h in range(8):
                    dst = ps_cs[h // 4][:, (h % 4) * 72:(h % 4 + 1) * 72]
                    for jt in range(4):
                        nc.tensor.matmul(
                            dst,
                            lhsT=sT_int[jt][:, h * 128:(h + 1) * 128],
                            rhs=cw_t[:, V0 + jt * 576 + h * 72:
                                     V0 + jt * 576 + (h + 1) * 72],
                            start=(jt == 0), stop=(jt == 3),
                        )
                inv_ih = spool.tile([128, 64], F32, tag="inv_ih")
                for h in range(8):
                    nc.vector.reciprocal(
                        inv_ih[:, h * 8:(h + 1) * 8],
                        ps_cs[h // 4][:, (h % 4) * 72 + 64:(h % 4) * 72 + 72])

                if lvl == 4:   # sums
                    dbg = spool.tile([128, 512], F32, tag="dbg")
                    nc.vector.tensor_copy(dbg[:, 0:64], inv_ih)
                    nc.vector.tensor_copy(dbg[:, 64:128], inv_ih)
                    nc.vector.memset(dbg[:, 128:512], 0.0)
                    nc.sync.dma_start(out=out[:, :], in_=dbg)

            if lvl >= 5:
                ctx_sb = spool.tile([128, 512], F32R, tag="ctx")
                for h in range(8):
                    nc.vector.tensor_scalar_mul(
                        ctx_sb[:, h * 64:(h + 1) * 64],
                        ps_cs[h // 4][:, (h % 4) * 72:(h % 4) * 72 + 64],
                        inv_ih[:, h * 8:h * 8 + 1])
                if lvl == 5:   # ctx
                    dbg = spool.tile([128, 512], F32, tag="dbg")
                    nc.vector.tensor_copy(dbg, ctx_sb)
                    nc.sync.dma_start(out=out[:, :], in_=dbg)

            if lvl >= 6:
                # ctxT
                ps_ct = psC.tile([128, 512], F32R, tag="ps_s", name="ps_ct")
                for dt_ in range(4):
                    nc.tensor.transpose(
                        out=ps_ct[:, dt_ * 128:(dt_ + 1) * 128],
                        in_=ctx_sb[:, dt_ * 128:(dt_ + 1) * 128],
                        identity=ident,
                    )
                ctxT_sb = spool.tile([128, 512], BF16, tag="ctxT")
                nc.vector.tensor_copy(ctxT_sb, ps_ct)
                # out projection
                ps_o = psB.tile([128, 512], F32, tag="pos", name="ps_o")
                for dt_ in range(4):
                    nc.tensor.matmul(
                        ps_o,
                        lhsT=ctxT_sb[:, dt_ * 128:(dt_ + 1) * 128],
                        rhs=cw_t[:, WO0 + dt_ * D:WO0 + (dt_ + 1) * D],
                        start=(dt_ == 0), stop=(dt_ == 3),
                    )
                out_sb = spool.tile([128, 512], F32, tag="out_sb")
                nc.vector.tensor_tensor(out_sb, ps_o, bo_bc,
                                        op=mybir.AluOpType.add)
                nc.sync.dma_start(out=out[:, :], in_=out_sb)

    nc.compile()
    return nc


def kernel(**inputs):
    inputs = {k: np.asarray(v) for k, v in inputs.items()}
    x = np.ascontiguousarray(inputs["inputs"], dtype=np.float32)      # [B, T, D]
    rel = inputs["rel_pos_emb"]                                        # [B, T, T, D]
    if rel.dtype != np.float32:
        rel = rel.astype(np.float32)
    f32 = lambda a: np.ascontiguousarray(a, dtype=np.float32)
    Wq, Wk, Wv, Wp, Wo = (f32(inputs[k]) for k in ("Wq", "Wk", "Wv", "Wp", "Wo"))
    bq, bk, bv, bp, bo = (f32(inputs[k]) for k in ("bq", "bk", "bv", "bp", "bo"))
    u = f32(inputs["u_bias"]).reshape(-1)
    v = f32(inputs["v_bias"]).reshape(-1)

    if "nc" not in _CACHED:
        _CACHED["nc"] = _build_nc()
    nc = _CACHED["nc"]

    SC = 1.0 / math.sqrt(HD)
    bf16 = ml_dtypes.bfloat16
    e3m4 = ml_dtypes.float8_e3m4

    def pack(w, ncol):
        # [rows, ncol] -> [p, (chunk, ncol)]: chunk-of-128-rows packing so
        # each tensor loads as a single long-run DMA
        return np.ascontiguousarray(
            np.asarray(w, np.float32).astype(bf16).reshape(
                -1, 128, ncol).transpose(1, 0, 2)).reshape(128, -1)

    # host-side projections (~1 GFLOP of numpy total)
    q_v = x @ Wq + bq + v                                # [B, T, D]
    q_u = (x @ Wq + bq + u) * SC
    k_all = x @ Wk + bk
    v_all = x @ Wv + bv
    Wp4 = Wp.reshape(D, H, HD)
    r_all = np.einsum("chd,bihd->bcih", Wp4,
                      q_v.reshape(B, T, H, HD) * SC).astype(bf16)
    wo_b = pack(Wo, D)

    in_maps = []
    for c in range(N_CORES):
        b, blk = c // 4, c % 4
        # rel shard: [128i, 512j, 512c] f32 -> e3m4, packed [p, g, i', ec, j]
        shard = rel[b, blk * I:(blk + 1) * I].astype(e3m4)
        shard = np.ascontiguousarray(
            shard.reshape(NG, GI, T, 4, 128).transpose(4, 0, 1, 3, 2)
        ).reshape(128, NG, GI * 4 * T)
        # r shard: [512c, 128i, 8h] -> [p, (ct, i, h)]
        r_shard = np.ascontiguousarray(
            r_all[b, :, blk * I:(blk + 1) * I, :].reshape(
                4, 128, I * 8).transpose(1, 0, 2)).reshape(128, 4 * I * 8)
        # v with 8 ones-columns per head: [4jm, 128p, 8h, 72]
        v4 = v_all[b].reshape(4, 128, H, HD)
        vo = np.concatenate(
            [v4, np.ones((4, 128, H, 8), np.float32)], axis=3)
        vp_b = np.ascontiguousarray(
            vo.astype(bf16).transpose(1, 0, 2, 3)).reshape(128, 4 * 8 * 72)
        cw_b = np.ascontiguousarray(np.concatenate(
            [pack(k_all[b].T, T),
             pack(q_u[b, blk * I:(blk + 1) * I].T, I),
             vp_b, wo_b], axis=1))
        in_maps.append({
            "rel": shard,
            "r": r_shard,
            "cw": cw_b,
            "bo": bo,
        })

    res = run_bass_kernel_spmd(nc, in_maps, list(range(N_CORES)),
                               trace=bool(os.environ.get("KBENCH_TRACE")),
                               tmpdir=os.environ.get("KBENCH_TMPDIR"))
    out = np.empty((B, T, D), np.float32)
    for c in range(N_CORES):
        b, blk = c // 4, c % 4
        out[b, blk * I:(blk + 1) * I] = res.results[c]["out"]
    if os.environ.get("KBENCH_TRACE"):
        _CACHED["last_exec_time_ns"] = res.exec_time_ns
        _CACHED["last_mean_exec_time_ns"] = res.mean_exec_time_ns
    return out
